# revision 20
# baseline (speedup 1.0000x reference)
"""BilinearInteraction Trainium2 kernel (8 NeuronCores, batch-sharded).

out[b, p=(i,j), d] = x[b, i, d] * (x @ W)[b, j, d]  for the 496 upper-tri
pairs of F=32 fields; x [4096, 32, 64] f32, W [64, 64] f32.

Roofline: the kernel is bound by DMA bytes through the per-NC HBM pipe
(~400 GB/s with all 8 cores active).  exec ~= preamble (~7 us) + all
DMA bytes / 400 GB/s + epilogue (~2.5 us), provided the SDMA pipe never
idles.  The pipeline therefore (a) runs everything in fp16 (rounding
~6e-4 relative, inside the 2e-2 gate; host upcasts to f32), halving
store traffic to 32.5 MB/core, (b) keeps load traffic minimal
(2.4 MB/core: fp16 x + a 0.125 MB host-pretransposed ramp slice), and
(c) orders work so stores can own the pipe the moment loads drain.

Per core: 512 batch rows, processed as 4 tiles of 128 (batch on SBUF
partitions). Per tile, in descending field order so the first-processed
output chunk only needs the tail of vid = x @ W:
  - ramp: vid fields 24-31 of tile 0 come from 4 PE matmuls on a HOST-
    pretransposed [128,512] XT slice (pure data marshalling) against a
    host-built block-diag [[W,0],[0,W]] (two fields per matmul), so the
    first TTs and the first store fire after one small load + 4 matmuls
    + 1 ACT copy -- no transpose chain on the critical path.
  - all other vid groups: PE pair-block transposes ([128,128] f-pair
    blocks -> PSUM) + matmuls, 4 f-pairs to a PSUM bank, ACT copies
    casting f32 PSUM -> fp16 SBUF.
  - pairwise Hadamard on DVE: one tensor_mul per field i covering all
    j>i at once, broadcasting x[:,i,:] over the j axis with a stride-0
    access pattern; innermost dim is 64 contiguous fp16 so the 2x_1P
    packed mode engages (2 elem/lane/cycle).
  - x arrives in 3 fat DMAs (tile-0 fields 24-31 first) into one
    [128, 8 KB/partition] SBUF tile: the HWDGE FIFO serializes per-DMA
    completion latency, so many small loads crawl at ~200 GB/s.
  - output staged in SBUF in 4 block-aligned chunks, each DMA'd as one
    ~2 MB contiguous-per-partition store on the sync HWDGE ring
    (measured gapless at ~400 GB/s; splitting stores across both HWDGE
    rings is SLOWER -- packet interleaving breaks HBM write locality).
    First/last chunks' stores are split so the stream starts early and
    the tail overlaps the final TTs.
"""

import sys

if "/opt/trn_rl_repo" not in sys.path:
    sys.path.insert(0, "/opt/trn_rl_repo")

import numpy as np

import concourse.bass as bass
import concourse.mybir as mybir
import concourse.tile as tile
from concourse import bacc
from concourse.bass_utils import run_bass_kernel_spmd

B, F, D = 4096, 32, 64
P = F * (F - 1) // 2  # 496
NCORES = 8
BSH = B // NCORES  # 512 batch rows per core
BT = 128  # batch tile (SBUF partitions)
NTILES = BSH // BT  # 4

f32 = mybir.dt.float32
f16 = mybir.dt.float16

FD = F * D  # 2048 cols per tile
G3 = 3 * (FD // 4)  # col 1536: fields 24-31 / f-pairs 12-15

# pair-block offsets: block i = pairs (i, j) for j in i+1..F-1
POFF = [0]
for i in range(F - 1):
    POFF.append(POFF[-1] + (F - 1 - i))
# chunk boundaries on block boundaries, ~4-way balanced: fat ~2 MB
# stores keep the sync ring near peak HBM bandwidth
CHUNKS = [(0, 171), (171, 343), (343, 496)]


def _emit(tc, nc, x_d, c_d, out_d):
    with (
        tc.tile_pool(name="const", bufs=1) as const_pool,
        tc.tile_pool(name="xall", bufs=1) as xall_pool,
        tc.tile_pool(name="xt3", bufs=1) as xt3_pool,
        tc.tile_pool(name="xtsb", bufs=4) as xtsb_pool,
        tc.tile_pool(name="vidp", bufs=2) as vid_pool,
        tc.tile_pool(name="outp", bufs=5) as out_pool,
        tc.tile_pool(name="ps_t", bufs=2, space="PSUM") as ps_t,
        tc.tile_pool(name="ps_m", bufs=2, space="PSUM") as ps_m,
    ):
        # ramp-critical loads, split across BOTH HWDGE rings so the
        # per-ring FIFO completion gaps overlap: the consts blob (XT
        # ramp slice + w2 + identity, one DMA) rides scalar while
        # tile-0 fields 24-31 of x ride sync (stores only need sync
        # from ~12 us on).  The group-3 vid matmuls + copy are emitted
        # BEFORE the bulk load issues so the scalar engine runs the
        # copy as soon as the matmuls finish instead of behind more
        # DMA-issue instructions.
        consts = const_pool.tile([128, 768], f16, tag="consts")
        nc.scalar.dma_start(out=consts[:], in_=c_d[:])
        xt3_t = consts[:, 0:512]
        w2 = consts[:, 512:640]
        ident = consts[:, 640:768]
        x_all = xall_pool.tile([128, NTILES * FD], f16, tag="xall")
        nc.sync.dma_start(
            out=x_all[:, G3:FD].rearrange("p (f d) -> p f d", d=D),
            in_=x_d[0:BT, 24:, :],
        )

        # vid tile 0, group 3 (fields 24-31): matmuls straight off the
        # host-pretransposed slice; first TTs depend only on this.
        vid0_t = vid_pool.tile([128, FD], f16, tag="vidt")
        vg3_ps = ps_m.tile([128, 512], f32, tag="vidps")
        for k in range(4):
            nc.tensor.matmul(
                vg3_ps[:, k * 128 : (k + 1) * 128],
                xt3_t[:, k * 128 : (k + 1) * 128],
                w2,
                start=True,
                stop=True,
            )
        nc.scalar.copy(vid0_t[:, G3:], vg3_ps[:])

        # bulk loads on scalar: rest of tile 0, then tiles 1-3 as one
        # 3-run fat DMA.
        nc.scalar.dma_start(
            out=x_all[:, :G3].rearrange("p (f d) -> p f d", d=D),
            in_=x_d[0:BT, :24, :],
        )
        nc.scalar.dma_start(
            out=x_all[:, FD:].rearrange("p (t f d) -> p t f d", f=F, d=D),
            in_=x_d[BT:, :, :].rearrange("(t p) f d -> p t f d", p=BT),
        )

        for t in range(NTILES):
            b0 = t * BT
            xc0 = t * FD
            x3 = x_all[:, xc0 : xc0 + FD].rearrange("p (f d) -> p f d", d=D)

            # vid groups via PE transpose + matmul (4 f-pairs per PSUM
            # bank): tile 0 skips group 3 (already computed above).
            if t == 0:
                vid_t = vid0_t
                groups = (2, 1, 0)
            else:
                vid_t = vid_pool.tile([128, FD], f16, tag="vidt")
                groups = (3, 2, 1, 0)
            for g in groups:
                xT_ps = ps_t.tile([128, 512], f16, tag="xtps")
                for k in range(4):
                    c = xc0 + (4 * g + k) * 128
                    nc.tensor.transpose(
                        xT_ps[:, k * 128 : (k + 1) * 128],
                        x_all[:, c : c + 128],
                        ident,
                    )
                xT_sb = xtsb_pool.tile([128, 512], f16, tag="xtsb")
                nc.scalar.copy(xT_sb[:], xT_ps[:])
                vid_ps = ps_m.tile([128, 512], f32, tag="vidps")
                for k in range(4):
                    nc.tensor.matmul(
                        vid_ps[:, k * 128 : (k + 1) * 128],
                        xT_sb[:, k * 128 : (k + 1) * 128],
                        w2,
                        start=True,
                        stop=True,
                    )
                nc.scalar.copy(vid_t[:, g * 512 : (g + 1) * 512], vid_ps[:])
            vid3 = vid_t[:].rearrange("p (f d) -> p f d", d=D)

            for ci, (c0, c1) in enumerate(reversed(CHUNKS)):
                npair = c1 - c0
                o_t = out_pool.tile([128, npair * D], f16, tag="outs")
                o3 = o_t[:].rearrange("p (q d) -> p q d", d=D)
                for i in reversed(range(F - 1)):
                    blk0, blk1 = POFF[i], POFF[i + 1]
                    lo, hi = max(blk0, c0), min(blk1, c1)
                    if lo >= hi:
                        continue
                    nj = hi - lo
                    j0 = i + 1 + (lo - blk0)
                    nc.vector.tensor_mul(
                        o3[:, lo - c0 : hi - c0, :],
                        x3[:, i : i + 1, :].broadcast_to((128, nj, D)),
                        vid3[:, j0 : j0 + nj, :],
                    )
                if t == 0 and ci == 0:
                    # first chunk streams in pieces so the first store
                    # fires as soon as the first blocks' TTs land
                    subs = (
                        (493, 496),
                        (482, 493),
                        (468, 482),
                        (451, 468),
                        (418, 451),
                        (376, 418),
                        (343, 376),
                    )
                elif t == NTILES - 1 and ci == len(CHUNKS) - 1:
                    # last chunk streams in two pieces so the tail store
                    # overlaps the final TTs instead of draining after
                    subs = ((86, 171), (31, 86), (0, 31))
                else:
                    subs = ((c0, c1),)
                for s0, s1 in subs:
                    nc.sync.dma_start(
                        out=out_d[b0 : b0 + BT, s0:s1, :],
                        in_=o3[:, s0 - c0 : s1 - c0, :],
                    )


def build_nc():
    nc = bacc.Bacc("TRN2", target_bir_lowering=False, debug=False)
    x_d = nc.dram_tensor("x", [BSH, F, D], f16, kind="ExternalInput")
    c_d = nc.dram_tensor("CONSTS", [128, 768], f16, kind="ExternalInput")
    out_d = nc.dram_tensor("out", [BSH, P, D], f16, kind="ExternalOutput")
    with tile.TileContext(nc) as tc:
        _emit(tc, nc, x_d.ap(), c_d.ap(), out_d.ap())
    nc.compile()
    return nc


_NC = None


def kernel(x: np.ndarray, W: np.ndarray, _trace=False, _trace_kwargs=None):
    global _NC
    if _NC is None:
        _NC = build_nc()
    x16 = np.ascontiguousarray(x, dtype=np.float16)
    w2 = np.zeros((128, 128), dtype=np.float16)
    w2[:64, :64] = W.astype(np.float16)
    w2[64:, 64:] = W.astype(np.float16)
    i128 = np.eye(128, dtype=np.float16)
    # host-pretransposed ramp slice: tile 0, fields 24-31, per core:
    # XT3[(h,d), (kp,b)] = x16[core*512 + b, 24 + 2*kp + h, d];
    # packed with w2 and the identity into one consts blob (one DMA).
    xt3_all = (
        x16.reshape(NCORES, NTILES, BT, F // 2, 2, D)[:, 0, :, 12:, :, :]
        .transpose(0, 3, 4, 2, 1)
        .reshape(NCORES, 128, 512)
    )
    consts = np.empty((NCORES, 128, 768), dtype=np.float16)
    consts[:, :, 0:512] = xt3_all
    consts[:, :, 512:640] = w2
    consts[:, :, 640:768] = i128
    in_maps = [
        {"x": x16[i * BSH : (i + 1) * BSH], "CONSTS": consts[i]}
        for i in range(NCORES)
    ]
    res = run_bass_kernel_spmd(
        _NC,
        in_maps,
        core_ids=list(range(NCORES)),
        trace=_trace,
        **(_trace_kwargs or {}),
    )
    out = np.concatenate(
        [res.results[i]["out"].astype(np.float32) for i in range(NCORES)], axis=0
    )
    if _trace:
        return out, res
    return out


# revision 21
# speedup vs baseline: 1.0108x; 1.0108x over previous
"""BilinearInteraction Trainium2 kernel (8 NeuronCores, batch-sharded).

out[b, p=(i,j), d] = x[b, i, d] * (x @ W)[b, j, d]  for the 496 upper-tri
pairs of F=32 fields; x [4096, 32, 64] f32, W [64, 64] f32.

Roofline: the kernel is bound by DMA bytes through the per-NC HBM pipe
(~400 GB/s with all 8 cores active).  exec ~= preamble (~7 us) + all
DMA bytes / 400 GB/s + epilogue (~2.5 us), provided the SDMA pipe never
idles.  The pipeline therefore (a) runs everything in fp16 (rounding
~6e-4 relative, inside the 2e-2 gate; host upcasts to f32), halving
store traffic to 32.5 MB/core, (b) keeps load traffic minimal
(2.4 MB/core: fp16 x + a 0.125 MB host-pretransposed ramp slice), and
(c) orders work so stores can own the pipe the moment loads drain.

Per core: 512 batch rows, processed as 4 tiles of 128 (batch on SBUF
partitions). Per tile, in descending field order so the first-processed
output chunk only needs the tail of vid = x @ W:
  - ramp: vid fields 24-31 of tile 0 come from 4 PE matmuls on a HOST-
    pretransposed [128,512] XT slice (pure data marshalling) against a
    host-built block-diag [[W,0],[0,W]] (two fields per matmul), so the
    first TTs and the first store fire after one small load + 4 matmuls
    + 1 ACT copy -- no transpose chain on the critical path.
  - all other vid groups: PE pair-block transposes ([128,128] f-pair
    blocks -> PSUM) + matmuls, 4 f-pairs to a PSUM bank, ACT copies
    casting f32 PSUM -> fp16 SBUF.
  - pairwise Hadamard on DVE: one tensor_mul per field i covering all
    j>i at once, broadcasting x[:,i,:] over the j axis with a stride-0
    access pattern; innermost dim is 64 contiguous fp16 so the 2x_1P
    packed mode engages (2 elem/lane/cycle).
  - x arrives in 3 fat DMAs (tile-0 fields 24-31 first) into one
    [128, 8 KB/partition] SBUF tile: the HWDGE FIFO serializes per-DMA
    completion latency, so many small loads crawl at ~200 GB/s.
  - output staged in SBUF in 4 block-aligned chunks, each DMA'd as one
    ~2 MB contiguous-per-partition store on the sync HWDGE ring
    (measured gapless at ~400 GB/s; splitting stores across both HWDGE
    rings is SLOWER -- packet interleaving breaks HBM write locality).
    First/last chunks' stores are split so the stream starts early and
    the tail overlaps the final TTs.
"""

import sys

if "/opt/trn_rl_repo" not in sys.path:
    sys.path.insert(0, "/opt/trn_rl_repo")

import numpy as np

import concourse.bass as bass
import concourse.mybir as mybir
import concourse.tile as tile
from concourse import bacc
from concourse.bass_utils import run_bass_kernel_spmd

B, F, D = 4096, 32, 64
P = F * (F - 1) // 2  # 496
NCORES = 8
BSH = B // NCORES  # 512 batch rows per core
BT = 128  # batch tile (SBUF partitions)
NTILES = BSH // BT  # 4

f32 = mybir.dt.float32
f16 = mybir.dt.float16

FD = F * D  # 2048 cols per tile
G3 = 3 * (FD // 4)  # col 1536: fields 24-31 / f-pairs 12-15

# pair-block offsets: block i = pairs (i, j) for j in i+1..F-1
POFF = [0]
for i in range(F - 1):
    POFF.append(POFF[-1] + (F - 1 - i))
# chunk boundaries on block boundaries, ~4-way balanced: fat ~2 MB
# stores keep the sync ring near peak HBM bandwidth
CHUNKS = [(0, 118), (118, 243), (243, 376), (376, 496)]


def _emit(tc, nc, x_d, c_d, out_d):
    with (
        tc.tile_pool(name="const", bufs=1) as const_pool,
        tc.tile_pool(name="xall", bufs=1) as xall_pool,
        tc.tile_pool(name="xt3", bufs=1) as xt3_pool,
        tc.tile_pool(name="xtsb", bufs=4) as xtsb_pool,
        tc.tile_pool(name="vidp", bufs=2) as vid_pool,
        tc.tile_pool(name="outp", bufs=6) as out_pool,
        tc.tile_pool(name="ps_t", bufs=2, space="PSUM") as ps_t,
        tc.tile_pool(name="ps_m", bufs=2, space="PSUM") as ps_m,
    ):
        # ramp-critical loads, split across BOTH HWDGE rings so the
        # per-ring FIFO completion gaps overlap: the consts blob (XT
        # ramp slice + w2 + identity, one DMA) rides scalar while
        # tile-0 fields 24-31 of x ride sync (stores only need sync
        # from ~12 us on).  The group-3 vid matmuls + copy are emitted
        # BEFORE the bulk load issues so the scalar engine runs the
        # copy as soon as the matmuls finish instead of behind more
        # DMA-issue instructions.
        consts = const_pool.tile([128, 768], f16, tag="consts")
        nc.scalar.dma_start(out=consts[:], in_=c_d[:])
        xt3_t = consts[:, 0:512]
        w2 = consts[:, 512:640]
        ident = consts[:, 640:768]
        x_all = xall_pool.tile([128, NTILES * FD], f16, tag="xall")
        nc.sync.dma_start(
            out=x_all[:, G3:FD].rearrange("p (f d) -> p f d", d=D),
            in_=x_d[0:BT, 24:, :],
        )

        # vid tile 0, group 3 (fields 24-31): matmuls straight off the
        # host-pretransposed slice; first TTs depend only on this.
        vid0_t = vid_pool.tile([128, FD], f16, tag="vidt")
        vg3_ps = ps_m.tile([128, 512], f32, tag="vidps")
        for k in range(4):
            nc.tensor.matmul(
                vg3_ps[:, k * 128 : (k + 1) * 128],
                xt3_t[:, k * 128 : (k + 1) * 128],
                w2,
                start=True,
                stop=True,
            )
        nc.scalar.copy(vid0_t[:, G3:], vg3_ps[:])

        # bulk loads on scalar: rest of tile 0, then tiles 1-3 as one
        # 3-run fat DMA.
        nc.scalar.dma_start(
            out=x_all[:, :G3].rearrange("p (f d) -> p f d", d=D),
            in_=x_d[0:BT, :24, :],
        )
        nc.scalar.dma_start(
            out=x_all[:, FD:].rearrange("p (t f d) -> p t f d", f=F, d=D),
            in_=x_d[BT:, :, :].rearrange("(t p) f d -> p t f d", p=BT),
        )

        for t in range(NTILES):
            b0 = t * BT
            xc0 = t * FD
            x3 = x_all[:, xc0 : xc0 + FD].rearrange("p (f d) -> p f d", d=D)

            # vid groups via PE transpose + matmul (4 f-pairs per PSUM
            # bank): tile 0 skips group 3 (already computed above).
            if t == 0:
                vid_t = vid0_t
                groups = (2, 1, 0)
            else:
                vid_t = vid_pool.tile([128, FD], f16, tag="vidt")
                groups = (3, 2, 1, 0)
            for g in groups:
                xT_ps = ps_t.tile([128, 512], f16, tag="xtps")
                for k in range(4):
                    c = xc0 + (4 * g + k) * 128
                    nc.tensor.transpose(
                        xT_ps[:, k * 128 : (k + 1) * 128],
                        x_all[:, c : c + 128],
                        ident,
                    )
                xT_sb = xtsb_pool.tile([128, 512], f16, tag="xtsb")
                nc.scalar.copy(xT_sb[:], xT_ps[:])
                vid_ps = ps_m.tile([128, 512], f32, tag="vidps")
                for k in range(4):
                    nc.tensor.matmul(
                        vid_ps[:, k * 128 : (k + 1) * 128],
                        xT_sb[:, k * 128 : (k + 1) * 128],
                        w2,
                        start=True,
                        stop=True,
                    )
                nc.scalar.copy(vid_t[:, g * 512 : (g + 1) * 512], vid_ps[:])
            vid3 = vid_t[:].rearrange("p (f d) -> p f d", d=D)

            for ci, (c0, c1) in enumerate(reversed(CHUNKS)):
                npair = c1 - c0
                o_t = out_pool.tile([128, npair * D], f16, tag="outs")
                o3 = o_t[:].rearrange("p (q d) -> p q d", d=D)
                for i in reversed(range(F - 1)):
                    blk0, blk1 = POFF[i], POFF[i + 1]
                    lo, hi = max(blk0, c0), min(blk1, c1)
                    if lo >= hi:
                        continue
                    nj = hi - lo
                    j0 = i + 1 + (lo - blk0)
                    nc.vector.tensor_mul(
                        o3[:, lo - c0 : hi - c0, :],
                        x3[:, i : i + 1, :].broadcast_to((128, nj, D)),
                        vid3[:, j0 : j0 + nj, :],
                    )
                if t == 0 and ci == 0:
                    # first chunk streams in pieces so the first store
                    # fires as soon as the first blocks' TTs land
                    subs = (
                        (493, 496),
                        (482, 493),
                        (468, 482),
                        (451, 468),
                        (418, 451),
                        (376, 418),
                    )
                elif t == NTILES - 1 and ci == len(CHUNKS) - 1:
                    # last chunk streams in two pieces so the tail store
                    # overlaps the final TTs instead of draining after
                    subs = ((31, 118), (0, 31))
                else:
                    subs = ((c0, c1),)
                for s0, s1 in subs:
                    nc.sync.dma_start(
                        out=out_d[b0 : b0 + BT, s0:s1, :],
                        in_=o3[:, s0 - c0 : s1 - c0, :],
                    )


def build_nc():
    nc = bacc.Bacc("TRN2", target_bir_lowering=False, debug=False)
    x_d = nc.dram_tensor("x", [BSH, F, D], f16, kind="ExternalInput")
    c_d = nc.dram_tensor("CONSTS", [128, 768], f16, kind="ExternalInput")
    out_d = nc.dram_tensor("out", [BSH, P, D], f16, kind="ExternalOutput")
    with tile.TileContext(nc) as tc:
        _emit(tc, nc, x_d.ap(), c_d.ap(), out_d.ap())
    nc.compile()
    return nc


_NC = None


def kernel(x: np.ndarray, W: np.ndarray, _trace=False, _trace_kwargs=None):
    global _NC
    if _NC is None:
        _NC = build_nc()
    x16 = np.ascontiguousarray(x, dtype=np.float16)
    w2 = np.zeros((128, 128), dtype=np.float16)
    w2[:64, :64] = W.astype(np.float16)
    w2[64:, 64:] = W.astype(np.float16)
    i128 = np.eye(128, dtype=np.float16)
    # host-pretransposed ramp slice: tile 0, fields 24-31, per core:
    # XT3[(h,d), (kp,b)] = x16[core*512 + b, 24 + 2*kp + h, d];
    # packed with w2 and the identity into one consts blob (one DMA).
    xt3_all = (
        x16.reshape(NCORES, NTILES, BT, F // 2, 2, D)[:, 0, :, 12:, :, :]
        .transpose(0, 3, 4, 2, 1)
        .reshape(NCORES, 128, 512)
    )
    consts = np.empty((NCORES, 128, 768), dtype=np.float16)
    consts[:, :, 0:512] = xt3_all
    consts[:, :, 512:640] = w2
    consts[:, :, 640:768] = i128
    in_maps = [
        {"x": x16[i * BSH : (i + 1) * BSH], "CONSTS": consts[i]}
        for i in range(NCORES)
    ]
    res = run_bass_kernel_spmd(
        _NC,
        in_maps,
        core_ids=list(range(NCORES)),
        trace=_trace,
        **(_trace_kwargs or {}),
    )
    out = np.concatenate(
        [res.results[i]["out"].astype(np.float32) for i in range(NCORES)], axis=0
    )
    if _trace:
        return out, res
    return out
